# revision 20
# baseline (speedup 1.0000x reference)
"""Causal self-attention (B=4, T=2048, D=1024, H=16) on 8 trn2 NeuronCores.

Sharding: Megatron-style tensor parallel over heads (TP=2) x data parallel
over batch (DP=4). Core c handles batch c//2 and head-group c%2 (8 heads).
Each core computes its QKV projection slice, causal attention for its 8
heads, and a partial output projection; the host sums the two TP partials
per batch and adds b_proj.

Performance structure (hw-measured):
- q/k projections run in fp8e4 DoubleRow (K=256 per pass, 2x PE rate);
  x and w_qk are quantized host-side (w scaled x32; un-scaled at the
  PSUM->SBUF bias-add).  q/k activations are stored fp8 (validated:
  max-err ~7e-3 of 2e-2 budget).  v / PV / out-proj stay fp16.
- score matmuls (64-dim contraction) row-packed: both heads run
  concurrently on PE array row-tiles (0,0)/(64,0)
- software-pipelined emission: QKV / V / out-projection matmul items
  interleave into the attention stream as fillers; scores(t+1) issue
  before PV(t) so the PE never stalls on the exp->PV dependency
- 1/sqrt(hd) applied inside exp (scale=0.125); causal masking post-exp
  via gpsimd affine_select; softmax denominator via ones-column in PV
- all inputs host-prepacked so every DMA is a contiguous >=0.5MB copy;
  the scalar ring carries no DMA (pure exp)
- output partials fp16; host sums the two TP halves + b_proj in fp32
"""
import sys

sys.path.insert(0, "/opt/trn_rl_repo")

import numpy as np
import ml_dtypes

import concourse.bass as bass
import concourse.tile as tile
from concourse import bacc, mybir
from concourse.bass_utils import run_bass_kernel_spmd

B, T, D, H = 4, 2048, 1024, 16
HD = 64            # head dim
HL = 8             # heads per core (TP=2)
DL = HL * HD       # 512 local qkv width
KCH = D // 128     # 8 contraction chunks (fp16)
JCH = D // 256     # 4 DoubleRow chunks (fp8)
TCH = T // 128     # 16 T chunks of 128
TB = T // 512      # 4 T blocks of 512
F8 = mybir.dt.float8e4
F16 = mybir.dt.float16
F32 = mybir.dt.float32
DR = mybir.MatmulPerfMode.DoubleRow
NF8 = ml_dtypes.float8_e4m3fn

_cache = {}


def _build():
    nc = bacc.Bacc("TRN2", target_bir_lowering=False, num_devices=8)

    # host-prepacked flat inputs (contiguous DMAs)
    xT16 = nc.dram_tensor("xT16", [128, TB * KCH * 512], F16,
                          kind="ExternalInput")
    xT8 = nc.dram_tensor("xT8", [128, TB * 2 * JCH * 512], F8,
                         kind="ExternalInput")
    # cc-major fp8 qk weights: [cc, i, j, 128] so the cc=0/4 slices the
    # prologue needs are contiguous 128KB prefixes
    wqk8 = nc.dram_tensor("wqk8", [128, 8 * 2 * JCH * 128], F8,
                          kind="ExternalInput")
    bqk = nc.dram_tensor("bqk", [128, 2 * DL // 128], F32,
                         kind="ExternalInput")
    wv = nc.dram_tensor("wv", [128, KCH * DL], F16, kind="ExternalInput")
    bv = nc.dram_tensor("bv", [1, DL], F32, kind="ExternalInput")
    wp = nc.dram_tensor("wp", [128, (DL // 128) * D], F16,
                        kind="ExternalInput")
    out = nc.dram_tensor("out", [T, D], F16, kind="ExternalOutput")

    GE = mybir.AluOpType.is_ge

    with tile.TileContext(nc) as tc:
        with (
            tc.tile_pool(name="const", bufs=1) as const,
            tc.tile_pool(name="acts", bufs=1) as acts,
            tc.tile_pool(name="work", bufs=4) as work,
            tc.tile_pool(name="small", bufs=3) as small,
            tc.tile_pool(name="outp", bufs=3) as outp,
            tc.tile_pool(name="psm", bufs=2, space="PSUM") as psm,
            tc.tile_pool(name="pss", bufs=2, space="PSUM") as pss,
            tc.tile_pool(name="psy", bufs=2, space="PSUM") as psy,
        ):
            # ---- input DMAs (contiguous, ordered for earliest QKV) ----
            # sync ring: fp8 qk-path first, then fp16 x blocks
            wqk8_fl = const.tile([128, 8 * 2 * JCH * 128], F8)
            CCW = 2 * JCH * 128  # 1024 bytes per cc group
            nc.sync.dma_start(out=wqk8_fl[:, 0:CCW], in_=wqk8[:, 0:CCW])
            nc.sync.dma_start(out=wqk8_fl[:, 4 * CCW:5 * CCW],
                              in_=wqk8[:, 4 * CCW:5 * CCW])
            xT8_fl = const.tile([128, TB * 2 * JCH * 512], F8)
            xT16_fl = const.tile([128, TB * KCH * 512], F16)
            nc.sync.dma_start(out=xT8_fl[:, 0:4096], in_=xT8[:, 0:4096])
            nc.sync.dma_start(out=xT16_fl[:, 0:4096], in_=xT16[:, 0:4096])
            nc.sync.dma_start(out=wqk8_fl[:, CCW:4 * CCW],
                              in_=wqk8[:, CCW:4 * CCW])
            nc.sync.dma_start(out=wqk8_fl[:, 5 * CCW:8 * CCW],
                              in_=wqk8[:, 5 * CCW:8 * CCW])
            for tb in range(1, TB):
                nc.sync.dma_start(
                    out=xT8_fl[:, tb * 4096:(tb + 1) * 4096],
                    in_=xT8[:, tb * 4096:(tb + 1) * 4096])
                nc.sync.dma_start(
                    out=xT16_fl[:, tb * 4096:(tb + 1) * 4096],
                    in_=xT16[:, tb * 4096:(tb + 1) * 4096])
            wqk8_sb = wqk8_fl.rearrange("p (cc i j m) -> p cc i j m",
                                        cc=8, i=2, j=JCH)
            xT8_sb = xT8_fl.rearrange("p (tb i j t) -> p tb i j t",
                                      tb=TB, i=2, j=JCH)
            xT16_sb = xT16_fl.rearrange("p (tb k t) -> p tb k t",
                                        tb=TB, k=KCH)
            # gpsimd (SWDGE) ring: biases + v/proj weights
            bqk_sb = const.tile([128, 2 * DL // 128], F32)
            nc.gpsimd.dma_start(out=bqk_sb, in_=bqk[:, :])
            wv_fl = const.tile([128, KCH * DL], F16)
            nc.gpsimd.dma_start(out=wv_fl, in_=wv[:, :])
            wv_sb = wv_fl.rearrange("p (k n) -> p k n", k=KCH)
            bv_sb = const.tile([1, DL], F32)
            nc.gpsimd.dma_start(out=bv_sb, in_=bv[:, :])
            bvb_sb = const.tile([128, DL], F32)
            nc.gpsimd.partition_broadcast(bvb_sb, bv_sb)
            wp_fl = const.tile([128, (DL // 128) * D], F16)
            nc.gpsimd.dma_start(out=wp_fl, in_=wp[:, :])
            wp_sb = wp_fl.rearrange("p (c n) -> p c n", c=DL // 128)

            # ---- persistent activations ----
            # qT/kT (fp8): pair c holds head 2c on partitions 0:64 and head
            # 2c+1 on 64:128 (row-packed 64-dim score contraction).
            qT_sb = [acts.tile([128, T], F8, name=f"qT{c}", tag=f"qT{c}")
                     for c in range(4)]
            kT_sb = [acts.tile([128, T], F8, name=f"kT{c}", tag=f"kT{c}")
                     for c in range(4)]
            vaug = [acts.tile([128, HL * (HD + 1)], F16, name=f"va{t}",
                              tag=f"va{t}") for t in range(TCH)]
            for t in range(TCH):
                va3 = vaug[t].rearrange("p (h c) -> p h c", c=HD + 1)
                nc.gpsimd.memset(va3[:, :, HD], 1.0)
            yT_sb = [acts.tile([128, T], F16, name=f"yT{c}", tag=f"yT{c}")
                     for c in range(4)]

            # ---- work item emitters ----
            def qkv_item(tb, cc):
                def emit():
                    ps = psm.tile([128, 512], F32, name="psQ", tag="psQ")
                    for j in range(JCH):
                        nc.tensor.matmul(
                            ps,
                            wqk8_sb[:, cc, :, j, :],
                            xT8_sb[:, tb, :, j, :],
                            start=(j == 0), stop=(j == JCH - 1),
                            perf_mode=DR,
                        )
                    dst = qT_sb[cc] if cc < 4 else kT_sb[cc - 4]
                    nc.vector.tensor_scalar(
                        out=dst[:, 512 * tb:512 * (tb + 1)],
                        in0=ps,
                        scalar1=1.0 / 32.0,
                        scalar2=bqk_sb[:, cc:cc + 1],
                        op0=mybir.AluOpType.mult,
                        op1=mybir.AluOpType.add,
                    )
                return emit

            def v_item(t):
                def emit():
                    ps = psm.tile([128, 512], F32, name="psQ", tag="psQ")
                    for k in range(KCH):
                        nc.tensor.matmul(
                            ps,
                            xT16_sb[:, t // 4, k, 128 * (t % 4):
                                    128 * (t % 4) + 128],
                            wv_sb[:, k, :],
                            start=(k == 0), stop=(k == KCH - 1),
                        )
                    ps3 = ps.rearrange("p (h d) -> p h d", d=HD)
                    bv3 = bvb_sb.rearrange("p (h d) -> p h d", d=HD)
                    va3 = vaug[t].rearrange("p (h c) -> p h c", c=HD + 1)
                    nc.vector.tensor_add(va3[:, :, 0:HD], ps3, bv3)
                return emit

            def proj_item(t, nb, ring=None):
                def emit():
                    ps = psm.tile([128, 512], F32, name="psQ", tag="psQ")
                    for c in range(DL // 128):
                        nc.tensor.matmul(
                            ps,
                            yT_sb[c][:, 128 * t:128 * (t + 1)],
                            wp_sb[:, c, 512 * nb:512 * (nb + 1)],
                            start=(c == 0), stop=(c == DL // 128 - 1),
                        )
                    ob = outp.tile([128, 512], F16, name="ob", tag="ob")
                    nc.vector.tensor_copy(ob, ps)
                    (ring or nc.sync).dma_start(
                        out=out[128 * t:128 * (t + 1),
                                512 * nb:512 * (nb + 1)],
                        in_=ob,
                    )
                return emit

            # ---- filler schedule: (emit_fn, min_block) ----
            fillers = []

            def add(fn, blk):
                fillers.append((fn, blk))

            for t in range(0, 4):
                add(v_item(t), (0, 0))
            add(qkv_item(0, 1), (0, 0)); add(qkv_item(0, 5), (0, 0))
            add(qkv_item(0, 2), (0, 0)); add(qkv_item(0, 6), (0, 0))
            add(qkv_item(0, 3), (0, 0)); add(qkv_item(0, 7), (0, 0))
            add(qkv_item(1, 0), (0, 0)); add(qkv_item(1, 4), (0, 0))
            for t in range(4, 8):
                add(v_item(t), (0, 0))
            add(qkv_item(1, 1), (0, 1)); add(qkv_item(1, 5), (0, 1))
            add(qkv_item(1, 2), (0, 2)); add(qkv_item(1, 6), (0, 2))
            add(qkv_item(1, 3), (0, 3)); add(qkv_item(1, 7), (0, 3))
            add(qkv_item(2, 0), (1, 0)); add(qkv_item(2, 4), (1, 0))
            for t in range(8, 12):
                add(v_item(t), (1, 0))
            add(qkv_item(2, 1), (1, 1)); add(qkv_item(2, 5), (1, 1))
            add(qkv_item(2, 2), (1, 2)); add(qkv_item(2, 6), (1, 2))
            add(qkv_item(2, 3), (1, 3)); add(qkv_item(2, 7), (1, 3))
            for t in range(0, 4):
                for nb in range(2):
                    add(proj_item(t, nb), (1, 0))
            add(qkv_item(3, 0), (2, 0)); add(qkv_item(3, 4), (2, 0))
            for t in range(12, 16):
                add(v_item(t), (2, 0))
            add(qkv_item(3, 1), (2, 1)); add(qkv_item(3, 5), (2, 1))
            add(qkv_item(3, 2), (2, 2)); add(qkv_item(3, 6), (2, 2))
            add(qkv_item(3, 3), (2, 3)); add(qkv_item(3, 7), (2, 3))
            for t in range(4, 8):
                for nb in range(2):
                    add(proj_item(t, nb), (2, 0))
            for t in range(8, 12):
                for nb in range(2):
                    add(proj_item(t, nb), (3, 0))

            state = {"popped": 0}
            need_before = {
                (0, 1): 6, (0, 2): 8, (0, 3): 10,
                (1, 0): 16, (1, 1): 18, (1, 2): 20, (1, 3): 22,
                (2, 0): 28, (2, 1): 30, (2, 2): 32, (2, 3): 34,
                (3, 0): 48, (3, 1): 50, (3, 2): 52, (3, 3): 54,
            }

            def blk_le(a, b):
                return a[0] < b[0] or (a[0] == b[0] and a[1] <= b[1])

            def pop_one(cur_blk):
                i = state["popped"]
                if i < len(fillers) and blk_le(fillers[i][1], cur_blk):
                    fillers[i][0]()
                    state["popped"] = i + 1
                    return True
                return False

            def pop_until(n, cur_blk):
                while state["popped"] < n:
                    if not pop_one(cur_blk):
                        break

            # ---- prologue: enough QKV for block (0,0) ----
            qkv_item(0, 0)()
            qkv_item(0, 4)()

            # ---- attention stream with filler pump ----
            def scores(q0, c, t):
                m = t - 4 * q0
                lo = 128 * m if m > 0 else 0
                ps_s = pss.tile([128, 1024], F32, name="psS", tag="psS")
                for p in range(2):
                    po = 64 * p
                    nc.tensor.matmul(
                        ps_s[:, 512 * p + lo:512 * (p + 1)],
                        kT_sb[c][po:po + 64, 128 * t:128 * (t + 1)],
                        qT_sb[c][po:po + 64, 512 * q0 + lo:512 * (q0 + 1)],
                        start=True, stop=True,
                        tile_position=(po, 0),
                    )
                es = work.tile([128, 1024], F16, name="es", tag="es")
                nc.scalar.activation(
                    out=es[:, lo:1024],
                    in_=ps_s[:, lo:1024],
                    func=mybir.ActivationFunctionType.Exp,
                    scale=0.125,
                )
                if m >= 0:
                    es3 = es.rearrange("p (h q) -> p h q", q=512)
                    reg = es3[:, :, lo:lo + 128]
                    nc.gpsimd.affine_select(
                        out=reg, in_=reg,
                        pattern=[[0, 2], [1, 128]],
                        compare_op=GE, fill=0.0,
                        base=0, channel_multiplier=-1,
                    )
                return es, lo

            blocks = [(q0, c) for q0 in range(4) for c in range(4)]
            flat = [(bi, q0, c, t)
                    for bi, (q0, c) in enumerate(blocks)
                    for t in range(4 * q0 + 4)]
            es_map = {}
            psys = {}

            def emit_scores_at(idx):
                bi, q0, c, t = flat[idx]
                if t == 0:
                    pop_until(need_before.get(blocks[bi], 0), blocks[bi])
                es_map[idx] = scores(q0, c, t)

            emit_scores_at(0)
            pace = {}
            tiles_after = {}
            rem = 0
            for bi in range(len(blocks) - 1, -1, -1):
                rem += 4 * blocks[bi][0] + 4
                tiles_after[bi] = rem
            for idx, (bi, q0, c, t) in enumerate(flat):
                ntiles = 4 * q0 + 4
                if t == 0:
                    nblk = blocks[bi + 1] if bi + 1 < len(blocks) else None
                    flush_tgt = need_before.get(nblk, len(fillers)) \
                        if nblk else len(fillers)
                    base = state["popped"]
                    # spread the remaining fillers evenly over the
                    # remaining tiles so they don't bunch at the tail
                    prop = base + ((len(fillers) - base) * ntiles
                                   + tiles_after[bi] - 1) // tiles_after[bi]
                    pace[bi] = (base, min(max(flush_tgt, prop),
                                          len(fillers)))
                    psys[bi] = [psy.tile([HD + 1, 512], F32, name="psY",
                                         tag="psY") for p in range(2)]
                if idx + 1 < len(flat):
                    emit_scores_at(idx + 1)
                base, tgt = pace[bi]
                want = base + ((t + 1) * (tgt - base) + ntiles - 1) // ntiles
                while state["popped"] < want:
                    if not pop_one(blocks[bi]):
                        break
                es, lo = es_map.pop(idx)
                ps_ys = psys[bi]
                for p in range(2):
                    h = 2 * c + p
                    nc.tensor.matmul(
                        ps_ys[p][:, lo:512],
                        vaug[t][:, (HD + 1) * h:(HD + 1) * (h + 1)],
                        es[:, 512 * p + lo:512 * (p + 1)],
                        start=(t == 0), stop=(t == ntiles - 1),
                    )
                if t == ntiles - 1:
                    # free the PV accumulators quickly: one copy to SBUF per
                    # head, then normalize off SBUF (doesn't gate next PV)
                    for p in range(2):
                        poff = 64 * p
                        ys = small.tile([HD + 1, 512], F32, name="ys",
                                        tag="ys", bufs=4)
                        nc.vector.tensor_copy(ys, ps_ys[p][0:HD + 1, :])
                        dn = small.tile([1, 512], F32, name="dn", tag="dn")
                        nc.vector.tensor_copy(dn, ys[HD:HD + 1, :])
                        rcr = small.tile([1, 512], F32, name="rcr",
                                         tag="rcr")
                        nc.vector.reciprocal_approx_fast(rcr, dn)
                        rcb = small.tile([64, 512], F32, name="rcb",
                                         tag="rcb")
                        nc.gpsimd.partition_broadcast(rcb, rcr)
                        nc.vector.tensor_mul(
                            yT_sb[c][poff:poff + 64,
                                     512 * q0:512 * (q0 + 1)],
                            ys[0:HD, :],
                            rcb,
                        )
                    del psys[bi]

            # ---- epilogue (out-DMAs on the idle scalar ring) ----
            pop_until(len(fillers), (3, 3))
            for t in range(12, 16):
                for nb in range(2):
                    proj_item(t, nb, ring=nc.scalar)()

    nc.finalize()
    return nc


def _enable_trace_hooks():
    """Inject antenv.axon_hooks + no-op artifact upload so that
    run_bass_kernel_spmd(trace=True) works under axon in this image."""
    import types
    import antenv

    if "antenv.axon_hooks" not in sys.modules:
        mod = types.ModuleType("antenv.axon_hooks")
        state = {"hook": None}
        mod.set_axon_ntff_profile_hook = lambda h: state.__setitem__("hook", h)
        mod.get_axon_ntff_profile_hook = lambda: state["hook"]
        sys.modules["antenv.axon_hooks"] = mod
        antenv.axon_hooks = mod
        from trn_agent_boot.trn_boot import _ntff_profile_via_ctypes

        mod.set_axon_ntff_profile_hook(
            _ntff_profile_via_ctypes("/opt/axon/libaxon_pjrt.so"))
    from concourse import bass_utils as bu

    bu.upload_artifacts = lambda tmpdir: str(tmpdir)


def kernel(x, w_attn, b_attn, w_proj, b_proj, _trace=False):
    x = np.asarray(x)
    w_attn = np.asarray(w_attn)
    b_attn = np.asarray(b_attn)
    w_proj = np.asarray(w_proj)
    b_proj = np.asarray(b_proj)

    if "nc" not in _cache:
        _cache["nc"] = _build()
    nc = _cache["nc"]

    f16 = np.float16

    in_maps = []
    for core in range(8):
        b, hg = core // 2, core % 2
        qs = slice(hg * DL, (hg + 1) * DL)
        ks = slice(D + hg * DL, D + (hg + 1) * DL)
        vs = slice(2 * D + hg * DL, 2 * D + (hg + 1) * DL)

        xTb = x[b].T  # [D, T]
        # fp16 x, packed [128, tb, k, 512]: row 128k+p, col 512tb+t
        x16 = np.ascontiguousarray(
            xTb.reshape(KCH, 128, TB, 512).transpose(1, 2, 0, 3)
        ).reshape(128, -1).astype(f16)
        # fp8 x, packed [128, tb, i, j, 512]: row 256j+128i+p
        x8 = np.ascontiguousarray(
            xTb.reshape(JCH, 2, 128, TB, 512).transpose(2, 3, 1, 0, 4)
        ).reshape(128, -1).astype(NF8)
        # fp8 w_qk (x32), packed cc-major [128, cc, i, j, m]:
        # element = w[256j+128i+p, 128cc+m]
        wqk_host = np.concatenate(
            [w_attn[:, qs], w_attn[:, ks]], axis=1) * 32.0
        w8 = np.ascontiguousarray(
            wqk_host.reshape(JCH, 2, 128, 8, 128).transpose(2, 3, 1, 0, 4)
        ).reshape(128, -1).astype(NF8)
        bqk_host = np.concatenate(
            [b_attn[qs], b_attn[ks]]).astype(np.float32)
        # fp16 wv packed [128, k, n]: row 128k+p
        wv_host = np.ascontiguousarray(
            w_attn[:, vs].reshape(KCH, 128, DL).transpose(1, 0, 2)
        ).reshape(128, -1).astype(f16)
        # fp16 wp packed [128, c, n]: row 128c+p of the local slice
        wp_host = np.ascontiguousarray(
            w_proj[hg * DL:(hg + 1) * DL, :].reshape(
                DL // 128, 128, D).transpose(1, 0, 2)
        ).reshape(128, -1).astype(f16)

        in_maps.append({
            "xT16": x16,
            "xT8": x8,
            "wqk8": w8,
            "bqk": np.ascontiguousarray(bqk_host.reshape(8, 128).T),
            "wv": wv_host,
            "bv": np.ascontiguousarray(b_attn[vs][None, :]).astype(np.float32),
            "wp": wp_host,
        })

    kwargs = {}
    if _trace:
        _enable_trace_hooks()
        kwargs = dict(trace=True, trace_cores=[0])
    res = run_bass_kernel_spmd(nc, in_maps, core_ids=list(range(8)), **kwargs)

    outp = np.empty((B, T, D), np.float32)
    for b in range(B):
        outp[b] = (res.results[2 * b]["out"].astype(np.float32)
                   + res.results[2 * b + 1]["out"].astype(np.float32))
    outp += b_proj.astype(np.float32)

    if _trace:
        print(f"HW exec time: {res.exec_time_ns} ns")
    return outp


# revision 21
# speedup vs baseline: 1.0131x; 1.0131x over previous
"""Causal self-attention (B=4, T=2048, D=1024, H=16) on 8 trn2 NeuronCores.

Sharding: Megatron-style tensor parallel over heads (TP=2) x data parallel
over batch (DP=4). Core c handles batch c//2 and head-group c%2 (8 heads).
Each core computes its QKV projection slice, causal attention for its 8
heads, and a partial output projection; the host sums the two TP partials
per batch and adds b_proj.

Performance structure (hw-measured):
- q/k projections run in fp8e4 DoubleRow (K=256 per pass, 2x PE rate);
  x and w_qk are quantized host-side (w scaled x32; un-scaled at the
  PSUM->SBUF bias-add).  q/k activations are stored fp8 (validated:
  max-err ~7e-3 of 2e-2 budget).  v / PV / out-proj stay fp16.
- score matmuls (64-dim contraction) row-packed: both heads run
  concurrently on PE array row-tiles (0,0)/(64,0)
- software-pipelined emission: QKV / V / out-projection matmul items
  interleave into the attention stream as fillers; scores(t+1) issue
  before PV(t) so the PE never stalls on the exp->PV dependency
- 1/sqrt(hd) applied inside exp (scale=0.125); causal masking post-exp
  via gpsimd affine_select; softmax denominator via ones-column in PV
- all inputs host-prepacked so every DMA is a contiguous >=0.5MB copy;
  the scalar ring carries no DMA (pure exp)
- output partials fp16; host sums the two TP halves + b_proj in fp32
"""
import sys

sys.path.insert(0, "/opt/trn_rl_repo")

import numpy as np
import ml_dtypes

import concourse.bass as bass
import concourse.tile as tile
from concourse import bacc, mybir
from concourse.bass_utils import run_bass_kernel_spmd

B, T, D, H = 4, 2048, 1024, 16
HD = 64            # head dim
HL = 8             # heads per core (TP=2)
DL = HL * HD       # 512 local qkv width
KCH = D // 128     # 8 contraction chunks (fp16)
JCH = D // 256     # 4 DoubleRow chunks (fp8)
TCH = T // 128     # 16 T chunks of 128
TB = T // 512      # 4 T blocks of 512
F8 = mybir.dt.float8e4
F16 = mybir.dt.float16
F32 = mybir.dt.float32
DR = mybir.MatmulPerfMode.DoubleRow
NF8 = ml_dtypes.float8_e4m3fn

_cache = {}


def _build():
    nc = bacc.Bacc("TRN2", target_bir_lowering=False, num_devices=8)

    # host-prepacked flat inputs (contiguous DMAs)
    xT16 = nc.dram_tensor("xT16", [128, TB * KCH * 512], F16,
                          kind="ExternalInput")
    xT8 = nc.dram_tensor("xT8", [128, TB * 2 * JCH * 512], F8,
                         kind="ExternalInput")
    # cc-major fp8 qk weights: [cc, i, j, 128] so the cc=0/4 slices the
    # prologue needs are contiguous 128KB prefixes
    wqk8 = nc.dram_tensor("wqk8", [128, 8 * 2 * JCH * 128], F8,
                          kind="ExternalInput")
    bqk = nc.dram_tensor("bqk", [128, 2 * DL // 128], F32,
                         kind="ExternalInput")
    wv = nc.dram_tensor("wv", [128, KCH * DL], F16, kind="ExternalInput")
    bv = nc.dram_tensor("bv", [1, DL], F32, kind="ExternalInput")
    wp = nc.dram_tensor("wp", [128, (DL // 128) * D], F16,
                        kind="ExternalInput")
    out = nc.dram_tensor("out", [T, D], F16, kind="ExternalOutput")

    GE = mybir.AluOpType.is_ge

    with tile.TileContext(nc) as tc:
        with (
            tc.tile_pool(name="const", bufs=1) as const,
            tc.tile_pool(name="acts", bufs=1) as acts,
            tc.tile_pool(name="work", bufs=4) as work,
            tc.tile_pool(name="small", bufs=3) as small,
            tc.tile_pool(name="outp", bufs=3) as outp,
            tc.tile_pool(name="psm", bufs=2, space="PSUM") as psm,
            tc.tile_pool(name="pss", bufs=2, space="PSUM") as pss,
            tc.tile_pool(name="psy", bufs=2, space="PSUM") as psy,
        ):
            # ---- input DMAs (contiguous, ordered for earliest QKV) ----
            # sync ring: fp8 qk-path first, then fp16 x blocks
            wqk8_fl = const.tile([128, 8 * 2 * JCH * 128], F8)
            CCW = 2 * JCH * 128  # 1024 bytes per cc group
            nc.sync.dma_start(out=wqk8_fl[:, 0:CCW], in_=wqk8[:, 0:CCW])
            nc.sync.dma_start(out=wqk8_fl[:, 4 * CCW:5 * CCW],
                              in_=wqk8[:, 4 * CCW:5 * CCW])
            xT8_fl = const.tile([128, TB * 2 * JCH * 512], F8)
            xT16_fl = const.tile([128, TB * KCH * 512], F16)
            nc.sync.dma_start(out=xT8_fl[:, 0:4096], in_=xT8[:, 0:4096])
            nc.sync.dma_start(out=xT16_fl[:, 0:4096], in_=xT16[:, 0:4096])
            nc.sync.dma_start(out=wqk8_fl[:, CCW:4 * CCW],
                              in_=wqk8[:, CCW:4 * CCW])
            nc.sync.dma_start(out=wqk8_fl[:, 5 * CCW:8 * CCW],
                              in_=wqk8[:, 5 * CCW:8 * CCW])
            for tb in range(1, TB):
                nc.sync.dma_start(
                    out=xT8_fl[:, tb * 4096:(tb + 1) * 4096],
                    in_=xT8[:, tb * 4096:(tb + 1) * 4096])
                nc.sync.dma_start(
                    out=xT16_fl[:, tb * 4096:(tb + 1) * 4096],
                    in_=xT16[:, tb * 4096:(tb + 1) * 4096])
            wqk8_sb = wqk8_fl.rearrange("p (cc i j m) -> p cc i j m",
                                        cc=8, i=2, j=JCH)
            xT8_sb = xT8_fl.rearrange("p (tb i j t) -> p tb i j t",
                                      tb=TB, i=2, j=JCH)
            xT16_sb = xT16_fl.rearrange("p (tb k t) -> p tb k t",
                                        tb=TB, k=KCH)
            # gpsimd (SWDGE) ring: biases + v/proj weights
            bqk_sb = const.tile([128, 2 * DL // 128], F32)
            nc.gpsimd.dma_start(out=bqk_sb, in_=bqk[:, :])
            wv_fl = const.tile([128, KCH * DL], F16)
            nc.gpsimd.dma_start(out=wv_fl, in_=wv[:, :])
            wv_sb = wv_fl.rearrange("p (k n) -> p k n", k=KCH)
            bv_sb = const.tile([1, DL], F32)
            nc.gpsimd.dma_start(out=bv_sb, in_=bv[:, :])
            bvb_sb = const.tile([128, DL], F32)
            nc.gpsimd.partition_broadcast(bvb_sb, bv_sb)
            wp_fl = const.tile([128, (DL // 128) * D], F16)
            nc.gpsimd.dma_start(out=wp_fl, in_=wp[:, :])
            wp_sb = wp_fl.rearrange("p (c n) -> p c n", c=DL // 128)

            # ---- persistent activations ----
            # qT/kT (fp8): pair c holds head 2c on partitions 0:64 and head
            # 2c+1 on 64:128 (row-packed 64-dim score contraction).
            qT_sb = [acts.tile([128, T], F8, name=f"qT{c}", tag=f"qT{c}")
                     for c in range(4)]
            kT_sb = [acts.tile([128, T], F8, name=f"kT{c}", tag=f"kT{c}")
                     for c in range(4)]
            vaug = [acts.tile([128, HL * (HD + 1)], F16, name=f"va{t}",
                              tag=f"va{t}") for t in range(TCH)]
            for t in range(TCH):
                va3 = vaug[t].rearrange("p (h c) -> p h c", c=HD + 1)
                nc.gpsimd.memset(va3[:, :, HD], 1.0)
            yT_sb = [acts.tile([128, T], F16, name=f"yT{c}", tag=f"yT{c}")
                     for c in range(4)]

            # ---- work item emitters ----
            def qkv_item(tb, cc):
                def emit():
                    ps = psm.tile([128, 512], F32, name="psQ", tag="psQ")
                    for j in range(JCH):
                        nc.tensor.matmul(
                            ps,
                            wqk8_sb[:, cc, :, j, :],
                            xT8_sb[:, tb, :, j, :],
                            start=(j == 0), stop=(j == JCH - 1),
                            perf_mode=DR,
                        )
                    dst = qT_sb[cc] if cc < 4 else kT_sb[cc - 4]
                    nc.vector.tensor_scalar(
                        out=dst[:, 512 * tb:512 * (tb + 1)],
                        in0=ps,
                        scalar1=1.0 / 32.0,
                        scalar2=bqk_sb[:, cc:cc + 1],
                        op0=mybir.AluOpType.mult,
                        op1=mybir.AluOpType.add,
                    )
                return emit

            def v_item(t):
                def emit():
                    ps = psm.tile([128, 512], F32, name="psQ", tag="psQ")
                    for k in range(KCH):
                        nc.tensor.matmul(
                            ps,
                            xT16_sb[:, t // 4, k, 128 * (t % 4):
                                    128 * (t % 4) + 128],
                            wv_sb[:, k, :],
                            start=(k == 0), stop=(k == KCH - 1),
                        )
                    ps3 = ps.rearrange("p (h d) -> p h d", d=HD)
                    bv3 = bvb_sb.rearrange("p (h d) -> p h d", d=HD)
                    va3 = vaug[t].rearrange("p (h c) -> p h c", c=HD + 1)
                    nc.vector.tensor_add(va3[:, :, 0:HD], ps3, bv3)
                return emit

            def proj_item(t, nb, ring=None):
                def emit():
                    ps = psm.tile([128, 512], F32, name="psQ", tag="psQ")
                    for c in range(DL // 128):
                        nc.tensor.matmul(
                            ps,
                            yT_sb[c][:, 128 * t:128 * (t + 1)],
                            wp_sb[:, c, 512 * nb:512 * (nb + 1)],
                            start=(c == 0), stop=(c == DL // 128 - 1),
                        )
                    ob = outp.tile([128, 512], F16, name="ob", tag="ob")
                    nc.vector.tensor_copy(ob, ps)
                    (ring or nc.sync).dma_start(
                        out=out[128 * t:128 * (t + 1),
                                512 * nb:512 * (nb + 1)],
                        in_=ob,
                    )
                return emit

            # ---- filler schedule: (emit_fn, min_block) ----
            fillers = []

            def add(fn, blk):
                fillers.append((fn, blk))

            for t in range(0, 4):
                add(v_item(t), (0, 0))
            add(qkv_item(0, 1), (0, 0)); add(qkv_item(0, 5), (0, 0))
            add(qkv_item(0, 2), (0, 0)); add(qkv_item(0, 6), (0, 0))
            add(qkv_item(0, 3), (0, 0)); add(qkv_item(0, 7), (0, 0))
            add(qkv_item(1, 0), (0, 0)); add(qkv_item(1, 4), (0, 0))
            for t in range(4, 8):
                add(v_item(t), (0, 0))
            add(qkv_item(1, 1), (0, 1)); add(qkv_item(1, 5), (0, 1))
            add(qkv_item(1, 2), (0, 2)); add(qkv_item(1, 6), (0, 2))
            add(qkv_item(1, 3), (0, 3)); add(qkv_item(1, 7), (0, 3))
            add(qkv_item(2, 0), (1, 0)); add(qkv_item(2, 4), (1, 0))
            for t in range(8, 12):
                add(v_item(t), (1, 0))
            add(qkv_item(2, 1), (1, 1)); add(qkv_item(2, 5), (1, 1))
            add(qkv_item(2, 2), (1, 2)); add(qkv_item(2, 6), (1, 2))
            add(qkv_item(2, 3), (1, 3)); add(qkv_item(2, 7), (1, 3))
            for t in range(0, 4):
                for nb in range(2):
                    add(proj_item(t, nb), (1, 0))
            add(qkv_item(3, 0), (2, 0)); add(qkv_item(3, 4), (2, 0))
            for t in range(12, 16):
                add(v_item(t), (2, 0))
            add(qkv_item(3, 1), (2, 1)); add(qkv_item(3, 5), (2, 1))
            add(qkv_item(3, 2), (2, 2)); add(qkv_item(3, 6), (2, 2))
            add(qkv_item(3, 3), (2, 3)); add(qkv_item(3, 7), (2, 3))
            for t in range(4, 8):
                for nb in range(2):
                    add(proj_item(t, nb), (2, 0))
            for t in range(8, 12):
                for nb in range(2):
                    add(proj_item(t, nb), (3, 0))

            state = {"popped": 0}
            need_before = {
                (0, 1): 6, (0, 2): 8, (0, 3): 10,
                (1, 0): 16, (1, 1): 18, (1, 2): 20, (1, 3): 22,
                (2, 0): 28, (2, 1): 30, (2, 2): 32, (2, 3): 34,
                (3, 0): 48, (3, 1): 50, (3, 2): 52, (3, 3): 54,
            }

            def blk_le(a, b):
                return a[0] < b[0] or (a[0] == b[0] and a[1] <= b[1])

            def pop_one(cur_blk):
                i = state["popped"]
                if i < len(fillers) and blk_le(fillers[i][1], cur_blk):
                    fillers[i][0]()
                    state["popped"] = i + 1
                    return True
                return False

            def pop_until(n, cur_blk):
                while state["popped"] < n:
                    if not pop_one(cur_blk):
                        break

            # ---- prologue: enough QKV for block (0,0) ----
            qkv_item(0, 0)()
            qkv_item(0, 4)()

            # ---- attention stream with filler pump ----
            def scores(q0, c, t):
                m = t - 4 * q0
                lo = 128 * m if m > 0 else 0
                ps_s = pss.tile([128, 1024], F32, name="psS", tag="psS")
                for p in range(2):
                    po = 64 * p
                    nc.tensor.matmul(
                        ps_s[:, 512 * p + lo:512 * (p + 1)],
                        kT_sb[c][po:po + 64, 128 * t:128 * (t + 1)],
                        qT_sb[c][po:po + 64, 512 * q0 + lo:512 * (q0 + 1)],
                        start=True, stop=True,
                        tile_position=(po, 0),
                    )
                es = work.tile([128, 1024], F16, name="es", tag="es")
                nc.scalar.activation(
                    out=es[:, lo:1024],
                    in_=ps_s[:, lo:1024],
                    func=mybir.ActivationFunctionType.Exp,
                    scale=0.125,
                )
                if m >= 0:
                    es3 = es.rearrange("p (h q) -> p h q", q=512)
                    reg = es3[:, :, lo:lo + 128]
                    nc.gpsimd.affine_select(
                        out=reg, in_=reg,
                        pattern=[[0, 2], [1, 128]],
                        compare_op=GE, fill=0.0,
                        base=0, channel_multiplier=-1,
                    )
                return es, lo

            blocks = [(q0, c) for q0 in range(4) for c in range(4)]
            flat = [(bi, q0, c, t)
                    for bi, (q0, c) in enumerate(blocks)
                    for t in range(4 * q0 + 4)]
            es_map = {}
            psys = {}

            def emit_scores_at(idx):
                bi, q0, c, t = flat[idx]
                if t == 0:
                    pop_until(need_before.get(blocks[bi], 0), blocks[bi])
                es_map[idx] = scores(q0, c, t)

            emit_scores_at(0)
            pace = {}
            tiles_after = {}
            rem = 0
            for bi in range(len(blocks) - 1, -1, -1):
                rem += 4 * blocks[bi][0] + 4
                tiles_after[bi] = rem
            for idx, (bi, q0, c, t) in enumerate(flat):
                ntiles = 4 * q0 + 4
                if t == 0:
                    nblk = blocks[bi + 1] if bi + 1 < len(blocks) else None
                    flush_tgt = need_before.get(nblk, len(fillers)) \
                        if nblk else len(fillers)
                    base = state["popped"]
                    # spread remaining fillers evenly only within the last
                    # q0 row (avoids the tail bunch without starving the
                    # ACT-lean early blocks)
                    if q0 == 3:
                        prop = base + ((len(fillers) - base) * ntiles
                                       + tiles_after[bi] - 1) \
                            // tiles_after[bi]
                        tgt = min(max(flush_tgt, prop), len(fillers))
                    else:
                        tgt = flush_tgt
                    pace[bi] = (base, tgt)
                    psys[bi] = [psy.tile([HD + 1, 512], F32, name="psY",
                                         tag="psY") for p in range(2)]
                if idx + 1 < len(flat):
                    emit_scores_at(idx + 1)
                base, tgt = pace[bi]
                want = base + ((t + 1) * (tgt - base) + ntiles - 1) // ntiles
                while state["popped"] < want:
                    if not pop_one(blocks[bi]):
                        break
                es, lo = es_map.pop(idx)
                ps_ys = psys[bi]
                for p in range(2):
                    h = 2 * c + p
                    nc.tensor.matmul(
                        ps_ys[p][:, lo:512],
                        vaug[t][:, (HD + 1) * h:(HD + 1) * (h + 1)],
                        es[:, 512 * p + lo:512 * (p + 1)],
                        start=(t == 0), stop=(t == ntiles - 1),
                    )
                if t == ntiles - 1:
                    # free the PV accumulators quickly: one copy to SBUF per
                    # head, then normalize off SBUF (doesn't gate next PV)
                    for p in range(2):
                        poff = 64 * p
                        ys = small.tile([HD + 1, 512], F32, name="ys",
                                        tag="ys", bufs=4)
                        nc.vector.tensor_copy(ys, ps_ys[p][0:HD + 1, :])
                        dn = small.tile([1, 512], F32, name="dn", tag="dn")
                        nc.vector.tensor_copy(dn, ys[HD:HD + 1, :])
                        rcr = small.tile([1, 512], F32, name="rcr",
                                         tag="rcr")
                        nc.vector.reciprocal_approx_fast(rcr, dn)
                        rcb = small.tile([64, 512], F32, name="rcb",
                                         tag="rcb")
                        nc.gpsimd.partition_broadcast(rcb, rcr)
                        nc.vector.tensor_mul(
                            yT_sb[c][poff:poff + 64,
                                     512 * q0:512 * (q0 + 1)],
                            ys[0:HD, :],
                            rcb,
                        )
                    del psys[bi]

            # ---- epilogue (out-DMAs on the idle scalar ring) ----
            pop_until(len(fillers), (3, 3))
            for t in range(12, 16):
                for nb in range(2):
                    proj_item(t, nb, ring=nc.scalar)()

    nc.finalize()
    return nc


def _enable_trace_hooks():
    """Inject antenv.axon_hooks + no-op artifact upload so that
    run_bass_kernel_spmd(trace=True) works under axon in this image."""
    import types
    import antenv

    if "antenv.axon_hooks" not in sys.modules:
        mod = types.ModuleType("antenv.axon_hooks")
        state = {"hook": None}
        mod.set_axon_ntff_profile_hook = lambda h: state.__setitem__("hook", h)
        mod.get_axon_ntff_profile_hook = lambda: state["hook"]
        sys.modules["antenv.axon_hooks"] = mod
        antenv.axon_hooks = mod
        from trn_agent_boot.trn_boot import _ntff_profile_via_ctypes

        mod.set_axon_ntff_profile_hook(
            _ntff_profile_via_ctypes("/opt/axon/libaxon_pjrt.so"))
    from concourse import bass_utils as bu

    bu.upload_artifacts = lambda tmpdir: str(tmpdir)


def kernel(x, w_attn, b_attn, w_proj, b_proj, _trace=False):
    x = np.asarray(x)
    w_attn = np.asarray(w_attn)
    b_attn = np.asarray(b_attn)
    w_proj = np.asarray(w_proj)
    b_proj = np.asarray(b_proj)

    if "nc" not in _cache:
        _cache["nc"] = _build()
    nc = _cache["nc"]

    f16 = np.float16

    in_maps = []
    for core in range(8):
        b, hg = core // 2, core % 2
        qs = slice(hg * DL, (hg + 1) * DL)
        ks = slice(D + hg * DL, D + (hg + 1) * DL)
        vs = slice(2 * D + hg * DL, 2 * D + (hg + 1) * DL)

        xTb = x[b].T  # [D, T]
        # fp16 x, packed [128, tb, k, 512]: row 128k+p, col 512tb+t
        x16 = np.ascontiguousarray(
            xTb.reshape(KCH, 128, TB, 512).transpose(1, 2, 0, 3)
        ).reshape(128, -1).astype(f16)
        # fp8 x, packed [128, tb, i, j, 512]: row 256j+128i+p
        x8 = np.ascontiguousarray(
            xTb.reshape(JCH, 2, 128, TB, 512).transpose(2, 3, 1, 0, 4)
        ).reshape(128, -1).astype(NF8)
        # fp8 w_qk (x32), packed cc-major [128, cc, i, j, m]:
        # element = w[256j+128i+p, 128cc+m]
        wqk_host = np.concatenate(
            [w_attn[:, qs], w_attn[:, ks]], axis=1) * 32.0
        w8 = np.ascontiguousarray(
            wqk_host.reshape(JCH, 2, 128, 8, 128).transpose(2, 3, 1, 0, 4)
        ).reshape(128, -1).astype(NF8)
        bqk_host = np.concatenate(
            [b_attn[qs], b_attn[ks]]).astype(np.float32)
        # fp16 wv packed [128, k, n]: row 128k+p
        wv_host = np.ascontiguousarray(
            w_attn[:, vs].reshape(KCH, 128, DL).transpose(1, 0, 2)
        ).reshape(128, -1).astype(f16)
        # fp16 wp packed [128, c, n]: row 128c+p of the local slice
        wp_host = np.ascontiguousarray(
            w_proj[hg * DL:(hg + 1) * DL, :].reshape(
                DL // 128, 128, D).transpose(1, 0, 2)
        ).reshape(128, -1).astype(f16)

        in_maps.append({
            "xT16": x16,
            "xT8": x8,
            "wqk8": w8,
            "bqk": np.ascontiguousarray(bqk_host.reshape(8, 128).T),
            "wv": wv_host,
            "bv": np.ascontiguousarray(b_attn[vs][None, :]).astype(np.float32),
            "wp": wp_host,
        })

    kwargs = {}
    if _trace:
        _enable_trace_hooks()
        kwargs = dict(trace=True, trace_cores=[0])
    res = run_bass_kernel_spmd(nc, in_maps, core_ids=list(range(8)), **kwargs)

    outp = np.empty((B, T, D), np.float32)
    for b in range(B):
        outp[b] = (res.results[2 * b]["out"].astype(np.float32)
                   + res.results[2 * b + 1]["out"].astype(np.float32))
    outp += b_proj.astype(np.float32)

    if _trace:
        print(f"HW exec time: {res.exec_time_ns} ns")
    return outp


# revision 22
# speedup vs baseline: 1.0168x; 1.0037x over previous
"""Causal self-attention (B=4, T=2048, D=1024, H=16) on 8 trn2 NeuronCores.

Sharding: Megatron-style tensor parallel over heads (TP=2) x data parallel
over batch (DP=4). Core c handles batch c//2 and head-group c%2 (8 heads).
Each core computes its QKV projection slice, causal attention for its 8
heads, and a partial output projection; the host sums the two TP partials
per batch and adds b_proj.

Performance structure (hw-measured):
- q/k projections run in fp8e4 DoubleRow (K=256 per pass, 2x PE rate);
  x and w_qk are quantized host-side (w scaled x32; un-scaled at the
  PSUM->SBUF bias-add).  q/k activations are stored fp8 (validated:
  max-err ~7e-3 of 2e-2 budget).  v / PV / out-proj stay fp16.
- score matmuls (64-dim contraction) row-packed: both heads run
  concurrently on PE array row-tiles (0,0)/(64,0)
- software-pipelined emission: QKV / V / out-projection matmul items
  interleave into the attention stream as fillers; scores(t+1) issue
  before PV(t) so the PE never stalls on the exp->PV dependency
- 1/sqrt(hd) applied inside exp (scale=0.125); causal masking post-exp
  via gpsimd affine_select; softmax denominator via ones-column in PV
- all inputs host-prepacked so every DMA is a contiguous >=0.5MB copy;
  the scalar ring carries no DMA (pure exp)
- output partials fp16; host sums the two TP halves + b_proj in fp32
"""
import sys

sys.path.insert(0, "/opt/trn_rl_repo")

import numpy as np
import ml_dtypes

import concourse.bass as bass
import concourse.tile as tile
from concourse import bacc, mybir
from concourse.bass_utils import run_bass_kernel_spmd

B, T, D, H = 4, 2048, 1024, 16
HD = 64            # head dim
HL = 8             # heads per core (TP=2)
DL = HL * HD       # 512 local qkv width
KCH = D // 128     # 8 contraction chunks (fp16)
JCH = D // 256     # 4 DoubleRow chunks (fp8)
TCH = T // 128     # 16 T chunks of 128
TB = T // 512      # 4 T blocks of 512
F8 = mybir.dt.float8e4
F16 = mybir.dt.float16
F32 = mybir.dt.float32
DR = mybir.MatmulPerfMode.DoubleRow
NF8 = ml_dtypes.float8_e4m3fn

_cache = {}


def _build():
    nc = bacc.Bacc("TRN2", target_bir_lowering=False, num_devices=8)

    # host-prepacked flat inputs (contiguous DMAs)
    xT16 = nc.dram_tensor("xT16", [128, TB * KCH * 512], F16,
                          kind="ExternalInput")
    xT8 = nc.dram_tensor("xT8", [128, TB * 2 * JCH * 512], F8,
                         kind="ExternalInput")
    # cc-major fp8 qk weights: [cc, i, j, 128] so the cc=0/4 slices the
    # prologue needs are contiguous 128KB prefixes
    wqk8 = nc.dram_tensor("wqk8", [128, 8 * 2 * JCH * 128], F8,
                          kind="ExternalInput")
    bqk = nc.dram_tensor("bqk", [128, 2 * DL // 128], F32,
                         kind="ExternalInput")
    wv = nc.dram_tensor("wv", [128, KCH * DL], F16, kind="ExternalInput")
    bv = nc.dram_tensor("bv", [1, DL], F32, kind="ExternalInput")
    wp = nc.dram_tensor("wp", [128, (DL // 128) * D], F16,
                        kind="ExternalInput")
    out = nc.dram_tensor("out", [T, D], F16, kind="ExternalOutput")

    GE = mybir.AluOpType.is_ge

    with tile.TileContext(nc) as tc:
        with (
            tc.tile_pool(name="const", bufs=1) as const,
            tc.tile_pool(name="acts", bufs=1) as acts,
            tc.tile_pool(name="work", bufs=4) as work,
            tc.tile_pool(name="small", bufs=3) as small,
            tc.tile_pool(name="outp", bufs=3) as outp,
            tc.tile_pool(name="psm", bufs=2, space="PSUM") as psm,
            tc.tile_pool(name="pss", bufs=2, space="PSUM") as pss,
            tc.tile_pool(name="psy", bufs=2, space="PSUM") as psy,
        ):
            # ---- input DMAs (contiguous, ordered for earliest QKV) ----
            # sync ring: fp8 qk-path first, then fp16 x blocks
            wqk8_fl = const.tile([128, 8 * 2 * JCH * 128], F8)
            CCW = 2 * JCH * 128  # 1024 bytes per cc group
            nc.sync.dma_start(out=wqk8_fl[:, 0:CCW], in_=wqk8[:, 0:CCW])
            nc.sync.dma_start(out=wqk8_fl[:, 4 * CCW:5 * CCW],
                              in_=wqk8[:, 4 * CCW:5 * CCW])
            xT8_fl = const.tile([128, TB * 2 * JCH * 512], F8)
            xT16_fl = const.tile([128, TB * KCH * 512], F16)
            nc.sync.dma_start(out=xT8_fl[:, 0:4096], in_=xT8[:, 0:4096])
            nc.sync.dma_start(out=xT16_fl[:, 0:4096], in_=xT16[:, 0:4096])
            nc.sync.dma_start(out=wqk8_fl[:, CCW:4 * CCW],
                              in_=wqk8[:, CCW:4 * CCW])
            nc.sync.dma_start(out=wqk8_fl[:, 5 * CCW:8 * CCW],
                              in_=wqk8[:, 5 * CCW:8 * CCW])
            for tb in range(1, TB):
                nc.sync.dma_start(
                    out=xT8_fl[:, tb * 4096:(tb + 1) * 4096],
                    in_=xT8[:, tb * 4096:(tb + 1) * 4096])
                nc.sync.dma_start(
                    out=xT16_fl[:, tb * 4096:(tb + 1) * 4096],
                    in_=xT16[:, tb * 4096:(tb + 1) * 4096])
            wqk8_sb = wqk8_fl.rearrange("p (cc i j m) -> p cc i j m",
                                        cc=8, i=2, j=JCH)
            xT8_sb = xT8_fl.rearrange("p (tb i j t) -> p tb i j t",
                                      tb=TB, i=2, j=JCH)
            xT16_sb = xT16_fl.rearrange("p (tb k t) -> p tb k t",
                                        tb=TB, k=KCH)
            # gpsimd (SWDGE) ring: biases + v/proj weights
            bqk_sb = const.tile([128, 2 * DL // 128], F32)
            nc.gpsimd.dma_start(out=bqk_sb, in_=bqk[:, :])
            wv_fl = const.tile([128, KCH * DL], F16)
            nc.gpsimd.dma_start(out=wv_fl, in_=wv[:, :])
            wv_sb = wv_fl.rearrange("p (k n) -> p k n", k=KCH)
            bv_sb = const.tile([1, DL], F32)
            nc.gpsimd.dma_start(out=bv_sb, in_=bv[:, :])
            bvb_sb = const.tile([128, DL], F32)
            nc.gpsimd.partition_broadcast(bvb_sb, bv_sb)
            wp_fl = const.tile([128, (DL // 128) * D], F16)
            nc.gpsimd.dma_start(out=wp_fl, in_=wp[:, :])
            wp_sb = wp_fl.rearrange("p (c n) -> p c n", c=DL // 128)

            # ---- persistent activations ----
            # qT/kT (fp8): pair c holds head 2c on partitions 0:64 and head
            # 2c+1 on 64:128 (row-packed 64-dim score contraction).
            qT_sb = [acts.tile([128, T], F8, name=f"qT{c}", tag=f"qT{c}")
                     for c in range(4)]
            kT_sb = [acts.tile([128, T], F8, name=f"kT{c}", tag=f"kT{c}")
                     for c in range(4)]
            vaug = [acts.tile([128, HL * (HD + 1)], F16, name=f"va{t}",
                              tag=f"va{t}") for t in range(TCH)]
            for t in range(TCH):
                va3 = vaug[t].rearrange("p (h c) -> p h c", c=HD + 1)
                nc.gpsimd.memset(va3[:, :, HD], 1.0)
            yT_sb = [acts.tile([128, T], F16, name=f"yT{c}", tag=f"yT{c}")
                     for c in range(4)]

            # ---- work item emitters ----
            def qkv_item(tb, cc):
                def emit():
                    ps = psm.tile([128, 512], F32, name="psQ", tag="psQ")
                    for j in range(JCH):
                        nc.tensor.matmul(
                            ps,
                            wqk8_sb[:, cc, :, j, :],
                            xT8_sb[:, tb, :, j, :],
                            start=(j == 0), stop=(j == JCH - 1),
                            perf_mode=DR,
                        )
                    dst = qT_sb[cc] if cc < 4 else kT_sb[cc - 4]
                    if tb < 2:
                        # early rounds: the scalar engine is exp-idle here;
                        # offload the PSUM drain to keep DVE/psm unclogged
                        nc.scalar.activation(
                            out=dst[:, 512 * tb:512 * (tb + 1)],
                            in_=ps,
                            func=mybir.ActivationFunctionType.Identity,
                            scale=1.0 / 32.0,
                            bias=bqk_sb[:, cc:cc + 1],
                        )
                    else:
                        nc.vector.tensor_scalar(
                            out=dst[:, 512 * tb:512 * (tb + 1)],
                            in0=ps,
                            scalar1=1.0 / 32.0,
                            scalar2=bqk_sb[:, cc:cc + 1],
                            op0=mybir.AluOpType.mult,
                            op1=mybir.AluOpType.add,
                        )
                return emit

            def v_item(t):
                def emit():
                    ps = psm.tile([128, 512], F32, name="psQ", tag="psQ")
                    for k in range(KCH):
                        nc.tensor.matmul(
                            ps,
                            xT16_sb[:, t // 4, k, 128 * (t % 4):
                                    128 * (t % 4) + 128],
                            wv_sb[:, k, :],
                            start=(k == 0), stop=(k == KCH - 1),
                        )
                    ps3 = ps.rearrange("p (h d) -> p h d", d=HD)
                    bv3 = bvb_sb.rearrange("p (h d) -> p h d", d=HD)
                    va3 = vaug[t].rearrange("p (h c) -> p h c", c=HD + 1)
                    nc.vector.tensor_add(va3[:, :, 0:HD], ps3, bv3)
                return emit

            def proj_item(t, nb, ring=None):
                def emit():
                    ps = psm.tile([128, 512], F32, name="psQ", tag="psQ")
                    for c in range(DL // 128):
                        nc.tensor.matmul(
                            ps,
                            yT_sb[c][:, 128 * t:128 * (t + 1)],
                            wp_sb[:, c, 512 * nb:512 * (nb + 1)],
                            start=(c == 0), stop=(c == DL // 128 - 1),
                        )
                    ob = outp.tile([128, 512], F16, name="ob", tag="ob")
                    nc.vector.tensor_copy(ob, ps)
                    (ring or nc.sync).dma_start(
                        out=out[128 * t:128 * (t + 1),
                                512 * nb:512 * (nb + 1)],
                        in_=ob,
                    )
                return emit

            # ---- filler schedule: (emit_fn, min_block) ----
            fillers = []

            def add(fn, blk):
                fillers.append((fn, blk))

            for t in range(0, 4):
                add(v_item(t), (0, 0))
            add(qkv_item(0, 1), (0, 0)); add(qkv_item(0, 5), (0, 0))
            add(qkv_item(0, 2), (0, 0)); add(qkv_item(0, 6), (0, 0))
            add(qkv_item(0, 3), (0, 0)); add(qkv_item(0, 7), (0, 0))
            add(qkv_item(1, 0), (0, 0)); add(qkv_item(1, 4), (0, 0))
            for t in range(4, 8):
                add(v_item(t), (0, 0))
            add(qkv_item(1, 1), (0, 1)); add(qkv_item(1, 5), (0, 1))
            add(qkv_item(1, 2), (0, 2)); add(qkv_item(1, 6), (0, 2))
            add(qkv_item(1, 3), (0, 3)); add(qkv_item(1, 7), (0, 3))
            add(qkv_item(2, 0), (1, 0)); add(qkv_item(2, 4), (1, 0))
            for t in range(8, 12):
                add(v_item(t), (1, 0))
            add(qkv_item(2, 1), (1, 1)); add(qkv_item(2, 5), (1, 1))
            add(qkv_item(2, 2), (1, 2)); add(qkv_item(2, 6), (1, 2))
            add(qkv_item(2, 3), (1, 3)); add(qkv_item(2, 7), (1, 3))
            for t in range(0, 4):
                for nb in range(2):
                    add(proj_item(t, nb), (1, 0))
            add(qkv_item(3, 0), (2, 0)); add(qkv_item(3, 4), (2, 0))
            for t in range(12, 16):
                add(v_item(t), (2, 0))
            add(qkv_item(3, 1), (2, 1)); add(qkv_item(3, 5), (2, 1))
            add(qkv_item(3, 2), (2, 2)); add(qkv_item(3, 6), (2, 2))
            add(qkv_item(3, 3), (2, 3)); add(qkv_item(3, 7), (2, 3))
            for t in range(4, 8):
                for nb in range(2):
                    add(proj_item(t, nb), (2, 0))
            for t in range(8, 12):
                for nb in range(2):
                    add(proj_item(t, nb), (3, 0))

            state = {"popped": 0}
            need_before = {
                (0, 1): 6, (0, 2): 8, (0, 3): 10,
                (1, 0): 16, (1, 1): 18, (1, 2): 20, (1, 3): 22,
                (2, 0): 28, (2, 1): 30, (2, 2): 32, (2, 3): 34,
                (3, 0): 48, (3, 1): 50, (3, 2): 52, (3, 3): 54,
            }

            def blk_le(a, b):
                return a[0] < b[0] or (a[0] == b[0] and a[1] <= b[1])

            def pop_one(cur_blk):
                i = state["popped"]
                if i < len(fillers) and blk_le(fillers[i][1], cur_blk):
                    fillers[i][0]()
                    state["popped"] = i + 1
                    return True
                return False

            def pop_until(n, cur_blk):
                while state["popped"] < n:
                    if not pop_one(cur_blk):
                        break

            # ---- prologue: enough QKV for block (0,0) ----
            qkv_item(0, 0)()
            qkv_item(0, 4)()

            # ---- attention stream with filler pump ----
            def scores(q0, c, t):
                m = t - 4 * q0
                lo = 128 * m if m > 0 else 0
                ps_s = pss.tile([128, 1024], F32, name="psS", tag="psS")
                for p in range(2):
                    po = 64 * p
                    nc.tensor.matmul(
                        ps_s[:, 512 * p + lo:512 * (p + 1)],
                        kT_sb[c][po:po + 64, 128 * t:128 * (t + 1)],
                        qT_sb[c][po:po + 64, 512 * q0 + lo:512 * (q0 + 1)],
                        start=True, stop=True,
                        tile_position=(po, 0),
                    )
                es = work.tile([128, 1024], F16, name="es", tag="es")
                nc.scalar.activation(
                    out=es[:, lo:1024],
                    in_=ps_s[:, lo:1024],
                    func=mybir.ActivationFunctionType.Exp,
                    scale=0.125,
                )
                if m >= 0:
                    es3 = es.rearrange("p (h q) -> p h q", q=512)
                    reg = es3[:, :, lo:lo + 128]
                    nc.gpsimd.affine_select(
                        out=reg, in_=reg,
                        pattern=[[0, 2], [1, 128]],
                        compare_op=GE, fill=0.0,
                        base=0, channel_multiplier=-1,
                    )
                return es, lo

            blocks = [(q0, c) for q0 in range(4) for c in range(4)]
            flat = [(bi, q0, c, t)
                    for bi, (q0, c) in enumerate(blocks)
                    for t in range(4 * q0 + 4)]
            es_map = {}
            psys = {}

            def emit_scores_at(idx):
                bi, q0, c, t = flat[idx]
                if t == 0:
                    pop_until(need_before.get(blocks[bi], 0), blocks[bi])
                es_map[idx] = scores(q0, c, t)

            emit_scores_at(0)
            pace = {}
            tiles_after = {}
            rem = 0
            for bi in range(len(blocks) - 1, -1, -1):
                rem += 4 * blocks[bi][0] + 4
                tiles_after[bi] = rem
            for idx, (bi, q0, c, t) in enumerate(flat):
                ntiles = 4 * q0 + 4
                if t == 0:
                    nblk = blocks[bi + 1] if bi + 1 < len(blocks) else None
                    flush_tgt = need_before.get(nblk, len(fillers)) \
                        if nblk else len(fillers)
                    base = state["popped"]
                    # spread remaining fillers evenly only within the last
                    # q0 row (avoids the tail bunch without starving the
                    # ACT-lean early blocks)
                    if q0 == 3:
                        prop = base + ((len(fillers) - base) * ntiles
                                       + tiles_after[bi] - 1) \
                            // tiles_after[bi]
                        tgt = min(max(flush_tgt, prop), len(fillers))
                    else:
                        tgt = flush_tgt
                    pace[bi] = (base, tgt)
                    psys[bi] = [psy.tile([HD + 1, 512], F32, name="psY",
                                         tag="psY") for p in range(2)]
                if idx + 1 < len(flat):
                    emit_scores_at(idx + 1)
                base, tgt = pace[bi]
                want = base + ((t + 1) * (tgt - base) + ntiles - 1) // ntiles
                while state["popped"] < want:
                    if not pop_one(blocks[bi]):
                        break
                es, lo = es_map.pop(idx)
                ps_ys = psys[bi]
                for p in range(2):
                    h = 2 * c + p
                    nc.tensor.matmul(
                        ps_ys[p][:, lo:512],
                        vaug[t][:, (HD + 1) * h:(HD + 1) * (h + 1)],
                        es[:, 512 * p + lo:512 * (p + 1)],
                        start=(t == 0), stop=(t == ntiles - 1),
                    )
                if t == ntiles - 1:
                    # free the PV accumulators quickly: one copy to SBUF per
                    # head, then normalize off SBUF (doesn't gate next PV)
                    for p in range(2):
                        poff = 64 * p
                        ys = small.tile([HD + 1, 512], F32, name="ys",
                                        tag="ys", bufs=4)
                        nc.vector.tensor_copy(ys, ps_ys[p][0:HD + 1, :])
                        dn = small.tile([1, 512], F32, name="dn", tag="dn")
                        nc.vector.tensor_copy(dn, ys[HD:HD + 1, :])
                        rcr = small.tile([1, 512], F32, name="rcr",
                                         tag="rcr")
                        nc.vector.reciprocal_approx_fast(rcr, dn)
                        rcb = small.tile([64, 512], F32, name="rcb",
                                         tag="rcb")
                        nc.gpsimd.partition_broadcast(rcb, rcr)
                        nc.vector.tensor_mul(
                            yT_sb[c][poff:poff + 64,
                                     512 * q0:512 * (q0 + 1)],
                            ys[0:HD, :],
                            rcb,
                        )
                    del psys[bi]

            # ---- epilogue (out-DMAs on the idle scalar ring) ----
            pop_until(len(fillers), (3, 3))
            for t in range(12, 16):
                for nb in range(2):
                    proj_item(t, nb, ring=nc.scalar)()

    nc.finalize()
    return nc


def _enable_trace_hooks():
    """Inject antenv.axon_hooks + no-op artifact upload so that
    run_bass_kernel_spmd(trace=True) works under axon in this image."""
    import types
    import antenv

    if "antenv.axon_hooks" not in sys.modules:
        mod = types.ModuleType("antenv.axon_hooks")
        state = {"hook": None}
        mod.set_axon_ntff_profile_hook = lambda h: state.__setitem__("hook", h)
        mod.get_axon_ntff_profile_hook = lambda: state["hook"]
        sys.modules["antenv.axon_hooks"] = mod
        antenv.axon_hooks = mod
        from trn_agent_boot.trn_boot import _ntff_profile_via_ctypes

        mod.set_axon_ntff_profile_hook(
            _ntff_profile_via_ctypes("/opt/axon/libaxon_pjrt.so"))
    from concourse import bass_utils as bu

    bu.upload_artifacts = lambda tmpdir: str(tmpdir)


def kernel(x, w_attn, b_attn, w_proj, b_proj, _trace=False):
    x = np.asarray(x)
    w_attn = np.asarray(w_attn)
    b_attn = np.asarray(b_attn)
    w_proj = np.asarray(w_proj)
    b_proj = np.asarray(b_proj)

    if "nc" not in _cache:
        _cache["nc"] = _build()
    nc = _cache["nc"]

    f16 = np.float16

    in_maps = []
    for core in range(8):
        b, hg = core // 2, core % 2
        qs = slice(hg * DL, (hg + 1) * DL)
        ks = slice(D + hg * DL, D + (hg + 1) * DL)
        vs = slice(2 * D + hg * DL, 2 * D + (hg + 1) * DL)

        xTb = x[b].T  # [D, T]
        # fp16 x, packed [128, tb, k, 512]: row 128k+p, col 512tb+t
        x16 = np.ascontiguousarray(
            xTb.reshape(KCH, 128, TB, 512).transpose(1, 2, 0, 3)
        ).reshape(128, -1).astype(f16)
        # fp8 x, packed [128, tb, i, j, 512]: row 256j+128i+p
        x8 = np.ascontiguousarray(
            xTb.reshape(JCH, 2, 128, TB, 512).transpose(2, 3, 1, 0, 4)
        ).reshape(128, -1).astype(NF8)
        # fp8 w_qk (x32), packed cc-major [128, cc, i, j, m]:
        # element = w[256j+128i+p, 128cc+m]
        wqk_host = np.concatenate(
            [w_attn[:, qs], w_attn[:, ks]], axis=1) * 32.0
        w8 = np.ascontiguousarray(
            wqk_host.reshape(JCH, 2, 128, 8, 128).transpose(2, 3, 1, 0, 4)
        ).reshape(128, -1).astype(NF8)
        bqk_host = np.concatenate(
            [b_attn[qs], b_attn[ks]]).astype(np.float32)
        # fp16 wv packed [128, k, n]: row 128k+p
        wv_host = np.ascontiguousarray(
            w_attn[:, vs].reshape(KCH, 128, DL).transpose(1, 0, 2)
        ).reshape(128, -1).astype(f16)
        # fp16 wp packed [128, c, n]: row 128c+p of the local slice
        wp_host = np.ascontiguousarray(
            w_proj[hg * DL:(hg + 1) * DL, :].reshape(
                DL // 128, 128, D).transpose(1, 0, 2)
        ).reshape(128, -1).astype(f16)

        in_maps.append({
            "xT16": x16,
            "xT8": x8,
            "wqk8": w8,
            "bqk": np.ascontiguousarray(bqk_host.reshape(8, 128).T),
            "wv": wv_host,
            "bv": np.ascontiguousarray(b_attn[vs][None, :]).astype(np.float32),
            "wp": wp_host,
        })

    kwargs = {}
    if _trace:
        _enable_trace_hooks()
        kwargs = dict(trace=True, trace_cores=[0])
    res = run_bass_kernel_spmd(nc, in_maps, core_ids=list(range(8)), **kwargs)

    outp = np.empty((B, T, D), np.float32)
    for b in range(B):
        outp[b] = (res.results[2 * b]["out"].astype(np.float32)
                   + res.results[2 * b + 1]["out"].astype(np.float32))
    outp += b_proj.astype(np.float32)

    if _trace:
        print(f"HW exec time: {res.exec_time_ns} ns")
    return outp


# revision 26
# speedup vs baseline: 1.0229x; 1.0060x over previous
"""Causal self-attention (B=4, T=2048, D=1024, H=16) on 8 trn2 NeuronCores.

Sharding: Megatron-style tensor parallel over heads (TP=2) x data parallel
over batch (DP=4). Core c handles batch c//2 and head-group c%2 (8 heads).
Each core computes its QKV projection slice, causal attention for its 8
heads, and a partial output projection; the host sums the two TP partials
per batch and adds b_proj.

Performance structure (hw-measured):
- q/k projections run in fp8e4 DoubleRow (K=256 per pass, 2x PE rate);
  x and w_qk are quantized host-side (w scaled x32; un-scaled at the
  PSUM->SBUF bias-add).  q/k activations are stored fp8 (validated:
  max-err ~7e-3 of 2e-2 budget).  v / PV / out-proj stay fp16.
- score matmuls (64-dim contraction) row-packed: both heads run
  concurrently on PE array row-tiles (0,0)/(64,0)
- software-pipelined emission: QKV / V / out-projection matmul items
  interleave into the attention stream as fillers; scores(t+1) issue
  before PV(t) so the PE never stalls on the exp->PV dependency
- 1/sqrt(hd) applied inside exp (scale=0.125); causal masking post-exp
  via gpsimd affine_select; softmax denominator via ones-column in PV
- all inputs host-prepacked so every DMA is a contiguous >=0.5MB copy;
  the scalar ring carries no DMA (pure exp)
- output partials fp16; host sums the two TP halves + b_proj in fp32
"""
import sys

sys.path.insert(0, "/opt/trn_rl_repo")

import numpy as np
import ml_dtypes

import concourse.bass as bass
import concourse.tile as tile
from concourse import bacc, mybir
from concourse.bass_utils import run_bass_kernel_spmd

B, T, D, H = 4, 2048, 1024, 16
HD = 64            # head dim
HL = 8             # heads per core (TP=2)
DL = HL * HD       # 512 local qkv width
KCH = D // 128     # 8 contraction chunks (fp16)
JCH = D // 256     # 4 DoubleRow chunks (fp8)
TCH = T // 128     # 16 T chunks of 128
TB = T // 512      # 4 T blocks of 512
F8 = mybir.dt.float8e4
F16 = mybir.dt.float16
F32 = mybir.dt.float32
DR = mybir.MatmulPerfMode.DoubleRow
NF8 = ml_dtypes.float8_e4m3fn

_cache = {}


def _build():
    nc = bacc.Bacc("TRN2", target_bir_lowering=False, num_devices=8)

    # host-prepacked flat inputs (contiguous DMAs)
    xT16 = nc.dram_tensor("xT16", [128, TB * KCH * 512], F16,
                          kind="ExternalInput")
    xT8 = nc.dram_tensor("xT8", [128, TB * 2 * JCH * 512], F8,
                         kind="ExternalInput")
    # cc-major fp8 qk weights: [cc, i, j, 128] so the cc=0/4 slices the
    # prologue needs are contiguous 128KB prefixes
    wqk8 = nc.dram_tensor("wqk8", [128, 8 * 2 * JCH * 128], F8,
                          kind="ExternalInput")
    bqk = nc.dram_tensor("bqk", [128, 2 * DL // 128], F32,
                         kind="ExternalInput")
    wv = nc.dram_tensor("wv", [128, KCH * DL], F16, kind="ExternalInput")
    bv = nc.dram_tensor("bv", [1, DL], F32, kind="ExternalInput")
    wp = nc.dram_tensor("wp", [128, (DL // 128) * D], F16,
                        kind="ExternalInput")
    out = nc.dram_tensor("out", [T, D], F16, kind="ExternalOutput")

    GE = mybir.AluOpType.is_ge

    with tile.TileContext(nc) as tc:
        with (
            tc.tile_pool(name="const", bufs=1) as const,
            tc.tile_pool(name="acts", bufs=1) as acts,
            tc.tile_pool(name="work", bufs=4) as work,
            tc.tile_pool(name="small", bufs=3) as small,
            tc.tile_pool(name="outp", bufs=3) as outp,
            tc.tile_pool(name="psm", bufs=2, space="PSUM") as psm,
            tc.tile_pool(name="pss", bufs=2, space="PSUM") as pss,
            tc.tile_pool(name="psy", bufs=2, space="PSUM") as psy,
        ):
            # ---- input DMAs (contiguous, ordered for earliest QKV) ----
            # sync ring: fp8 qk-path first, then fp16 x blocks
            wqk8_fl = const.tile([128, 8 * 2 * JCH * 128], F8)
            CCW = 2 * JCH * 128  # 1024 bytes per cc group
            nc.sync.dma_start(out=wqk8_fl[:, 0:CCW], in_=wqk8[:, 0:CCW])
            nc.sync.dma_start(out=wqk8_fl[:, 4 * CCW:5 * CCW],
                              in_=wqk8[:, 4 * CCW:5 * CCW])
            xT8_fl = const.tile([128, TB * 2 * JCH * 512], F8)
            xT16_fl = const.tile([128, TB * KCH * 512], F16)
            nc.sync.dma_start(out=xT8_fl[:, 0:4096], in_=xT8[:, 0:4096])
            nc.sync.dma_start(out=xT16_fl[:, 0:4096], in_=xT16[:, 0:4096])
            nc.sync.dma_start(out=wqk8_fl[:, CCW:4 * CCW],
                              in_=wqk8[:, CCW:4 * CCW])
            nc.sync.dma_start(out=wqk8_fl[:, 5 * CCW:8 * CCW],
                              in_=wqk8[:, 5 * CCW:8 * CCW])
            for tb in range(1, TB):
                nc.sync.dma_start(
                    out=xT8_fl[:, tb * 4096:(tb + 1) * 4096],
                    in_=xT8[:, tb * 4096:(tb + 1) * 4096])
                nc.sync.dma_start(
                    out=xT16_fl[:, tb * 4096:(tb + 1) * 4096],
                    in_=xT16[:, tb * 4096:(tb + 1) * 4096])
            wqk8_sb = wqk8_fl.rearrange("p (cc i j m) -> p cc i j m",
                                        cc=8, i=2, j=JCH)
            xT8_sb = xT8_fl.rearrange("p (tb i j t) -> p tb i j t",
                                      tb=TB, i=2, j=JCH)
            xT16_sb = xT16_fl.rearrange("p (tb k t) -> p tb k t",
                                        tb=TB, k=KCH)
            # gpsimd (SWDGE) ring: biases + v/proj weights
            bqk_sb = const.tile([128, 2 * DL // 128], F32)
            nc.gpsimd.dma_start(out=bqk_sb, in_=bqk[:, :])
            wv_fl = const.tile([128, KCH * DL], F16)
            nc.gpsimd.dma_start(out=wv_fl, in_=wv[:, :])
            wv_sb = wv_fl.rearrange("p (k n) -> p k n", k=KCH)
            bv_sb = const.tile([1, DL], F32)
            nc.gpsimd.dma_start(out=bv_sb, in_=bv[:, :])
            bvb_sb = const.tile([128, DL], F32)
            nc.gpsimd.partition_broadcast(bvb_sb, bv_sb)
            wp_fl = const.tile([128, (DL // 128) * D], F16)
            nc.gpsimd.dma_start(out=wp_fl, in_=wp[:, :])
            wp_sb = wp_fl.rearrange("p (c n) -> p c n", c=DL // 128)

            # ---- persistent activations ----
            # qT/kT (fp8): pair c holds head 2c on partitions 0:64 and head
            # 2c+1 on 64:128 (row-packed 64-dim score contraction).
            qT_sb = [acts.tile([128, T], F8, name=f"qT{c}", tag=f"qT{c}")
                     for c in range(4)]
            kT_sb = [acts.tile([128, T], F8, name=f"kT{c}", tag=f"kT{c}")
                     for c in range(4)]
            vaug = [acts.tile([128, HL * (HD + 1)], F16, name=f"va{t}",
                              tag=f"va{t}") for t in range(TCH)]
            for t in range(TCH):
                va3 = vaug[t].rearrange("p (h c) -> p h c", c=HD + 1)
                nc.gpsimd.memset(va3[:, :, HD], 1.0)
            yT_sb = [acts.tile([128, T], F16, name=f"yT{c}", tag=f"yT{c}")
                     for c in range(4)]

            # ---- work item emitters ----
            def qkv_item(tb, cc):
                def emit():
                    ps = psm.tile([128, 512], F32, name="psQ", tag="psQ")
                    for j in range(JCH):
                        nc.tensor.matmul(
                            ps,
                            wqk8_sb[:, cc, :, j, :],
                            xT8_sb[:, tb, :, j, :],
                            start=(j == 0), stop=(j == JCH - 1),
                            perf_mode=DR,
                        )
                    dst = qT_sb[cc] if cc < 4 else kT_sb[cc - 4]
                    if tb < 2:
                        # early rounds: the scalar engine is exp-idle here;
                        # offload the PSUM drain to keep DVE/psm unclogged
                        nc.scalar.activation(
                            out=dst[:, 512 * tb:512 * (tb + 1)],
                            in_=ps,
                            func=mybir.ActivationFunctionType.Identity,
                            scale=1.0 / 32.0,
                            bias=bqk_sb[:, cc:cc + 1],
                        )
                    else:
                        nc.vector.tensor_scalar(
                            out=dst[:, 512 * tb:512 * (tb + 1)],
                            in0=ps,
                            scalar1=1.0 / 32.0,
                            scalar2=bqk_sb[:, cc:cc + 1],
                            op0=mybir.AluOpType.mult,
                            op1=mybir.AluOpType.add,
                        )
                return emit

            def v_item(t):
                def emit():
                    ps = psm.tile([128, 512], F32, name="psQ", tag="psQ")
                    for k in range(KCH):
                        nc.tensor.matmul(
                            ps,
                            xT16_sb[:, t // 4, k, 128 * (t % 4):
                                    128 * (t % 4) + 128],
                            wv_sb[:, k, :],
                            start=(k == 0), stop=(k == KCH - 1),
                        )
                    ps3 = ps.rearrange("p (h d) -> p h d", d=HD)
                    bv3 = bvb_sb.rearrange("p (h d) -> p h d", d=HD)
                    va3 = vaug[t].rearrange("p (h c) -> p h c", c=HD + 1)
                    nc.vector.tensor_add(va3[:, :, 0:HD], ps3, bv3)
                return emit

            def proj_item(t, nb, ring=None):
                def emit():
                    ps = psm.tile([128, 512], F32, name="psQ", tag="psQ")
                    for c in range(DL // 128):
                        nc.tensor.matmul(
                            ps,
                            yT_sb[c][:, 128 * t:128 * (t + 1)],
                            wp_sb[:, c, 512 * nb:512 * (nb + 1)],
                            start=(c == 0), stop=(c == DL // 128 - 1),
                        )
                    ob = outp.tile([128, 512], F16, name="ob", tag="ob")
                    nc.vector.tensor_copy(ob, ps)
                    (ring or nc.sync).dma_start(
                        out=out[128 * t:128 * (t + 1),
                                512 * nb:512 * (nb + 1)],
                        in_=ob,
                    )
                return emit

            # ---- filler schedule: (emit_fn, min_block) ----
            fillers = []

            def add(fn, blk):
                fillers.append((fn, blk))

            for t in range(0, 4):
                add(v_item(t), (0, 0))
            add(qkv_item(0, 1), (0, 0)); add(qkv_item(0, 5), (0, 0))
            add(qkv_item(0, 2), (0, 0)); add(qkv_item(0, 6), (0, 0))
            add(qkv_item(0, 3), (0, 0)); add(qkv_item(0, 7), (0, 0))
            add(qkv_item(1, 0), (0, 0)); add(qkv_item(1, 4), (0, 0))
            for t in range(4, 8):
                add(v_item(t), (0, 0))
            add(qkv_item(1, 1), (0, 1)); add(qkv_item(1, 5), (0, 1))
            add(qkv_item(1, 2), (0, 2)); add(qkv_item(1, 6), (0, 2))
            add(qkv_item(1, 3), (0, 3)); add(qkv_item(1, 7), (0, 3))
            add(qkv_item(2, 0), (1, 0)); add(qkv_item(2, 4), (1, 0))
            for t in range(8, 12):
                add(v_item(t), (1, 0))
            add(qkv_item(2, 1), (1, 1)); add(qkv_item(2, 5), (1, 1))
            add(qkv_item(2, 2), (1, 2)); add(qkv_item(2, 6), (1, 2))
            add(qkv_item(2, 3), (1, 3)); add(qkv_item(2, 7), (1, 3))
            for t in range(0, 4):
                for nb in range(2):
                    add(proj_item(t, nb), (1, 0))
            add(qkv_item(3, 0), (2, 0)); add(qkv_item(3, 4), (2, 0))
            for t in range(12, 16):
                add(v_item(t), (2, 0))
            add(qkv_item(3, 1), (2, 1)); add(qkv_item(3, 5), (2, 1))
            add(qkv_item(3, 2), (2, 2)); add(qkv_item(3, 6), (2, 2))
            add(qkv_item(3, 3), (2, 3)); add(qkv_item(3, 7), (2, 3))
            for t in range(4, 8):
                for nb in range(2):
                    add(proj_item(t, nb), (2, 0))
            for t in range(8, 12):
                for nb in range(2):
                    add(proj_item(t, nb), (3, 0))

            state = {"popped": 0}
            need_before = {
                (0, 1): 6, (0, 2): 8, (0, 3): 10,
                (1, 0): 16, (1, 1): 18, (1, 2): 20, (1, 3): 22,
                (2, 0): 28, (2, 1): 30, (2, 2): 32, (2, 3): 34,
                (3, 0): 48, (3, 1): 50, (3, 2): 52, (3, 3): 54,
            }

            def blk_le(a, b):
                return a[0] < b[0] or (a[0] == b[0] and a[1] <= b[1])

            def pop_one(cur_blk):
                i = state["popped"]
                if i < len(fillers) and blk_le(fillers[i][1], cur_blk):
                    fillers[i][0]()
                    state["popped"] = i + 1
                    return True
                return False

            def pop_until(n, cur_blk):
                while state["popped"] < n:
                    if not pop_one(cur_blk):
                        break

            # ---- prologue: enough QKV for block (0,0) ----
            qkv_item(0, 0)()
            qkv_item(0, 4)()

            # ---- attention stream with filler pump ----
            def scores(q0, c, t):
                m = t - 4 * q0
                lo = 128 * m if m > 0 else 0
                ps_s = pss.tile([128, 1024], F32, name="psS", tag="psS")
                for p in range(2):
                    po = 64 * p
                    nc.tensor.matmul(
                        ps_s[:, 512 * p + lo:512 * (p + 1)],
                        kT_sb[c][po:po + 64, 128 * t:128 * (t + 1)],
                        qT_sb[c][po:po + 64, 512 * q0 + lo:512 * (q0 + 1)],
                        start=True, stop=True,
                        tile_position=(po, 0),
                    )
                es = work.tile([128, 1024], F16, name="es", tag="es")
                nc.scalar.activation(
                    out=es[:, lo:1024],
                    in_=ps_s[:, lo:1024],
                    func=mybir.ActivationFunctionType.Exp,
                    scale=0.125,
                )
                if m >= 0:
                    es3 = es.rearrange("p (h q) -> p h q", q=512)
                    reg = es3[:, :, lo:lo + 128]
                    nc.gpsimd.affine_select(
                        out=reg, in_=reg,
                        pattern=[[0, 2], [1, 128]],
                        compare_op=GE, fill=0.0,
                        base=0, channel_multiplier=-1,
                    )
                return es, lo

            blocks = [(q0, c) for q0 in range(4) for c in range(4)]
            flat = [(bi, q0, c, t)
                    for bi, (q0, c) in enumerate(blocks)
                    for t in range(4 * q0 + 4)]
            es_map = {}
            psys = {}

            def emit_scores_at(idx):
                bi, q0, c, t = flat[idx]
                if t == 0:
                    pop_until(need_before.get(blocks[bi], 0), blocks[bi])
                es_map[idx] = scores(q0, c, t)

            emit_scores_at(0)
            pace = {}
            tiles_after = {}
            rem = 0
            for bi in range(len(blocks) - 1, -1, -1):
                rem += 4 * blocks[bi][0] + 4
                tiles_after[bi] = rem
            for idx, (bi, q0, c, t) in enumerate(flat):
                ntiles = 4 * q0 + 4
                if t == 0:
                    nblk = blocks[bi + 1] if bi + 1 < len(blocks) else None
                    flush_tgt = need_before.get(nblk, len(fillers)) \
                        if nblk else len(fillers)
                    base = state["popped"]
                    # spread remaining fillers evenly only within the last
                    # q0 row (avoids the tail bunch without starving the
                    # ACT-lean early blocks)
                    if q0 == 3:
                        prop = base + ((len(fillers) - base) * ntiles
                                       + tiles_after[bi] - 1) \
                            // tiles_after[bi]
                        tgt = min(max(flush_tgt, prop), len(fillers))
                    else:
                        tgt = flush_tgt
                    pace[bi] = (base, tgt)
                    psys[bi] = [psy.tile([HD + 1, 512], F32, name="psY",
                                         tag="psY") for p in range(2)]
                if idx + 1 < len(flat):
                    emit_scores_at(idx + 1)
                base, tgt = pace[bi]
                want = base + ((t + 1) * (tgt - base) + ntiles - 1) // ntiles
                while state["popped"] < want:
                    if not pop_one(blocks[bi]):
                        break
                es, lo = es_map.pop(idx)
                ps_ys = psys[bi]
                for p in range(2):
                    h = 2 * c + p
                    nc.tensor.matmul(
                        ps_ys[p][:, lo:512],
                        vaug[t][:, (HD + 1) * h:(HD + 1) * (h + 1)],
                        es[:, 512 * p + lo:512 * (p + 1)],
                        start=(t == 0), stop=(t == ntiles - 1),
                    )
                if t == ntiles - 1:
                    # free the PV accumulators quickly: one copy to SBUF per
                    # head, then normalize off SBUF (doesn't gate next PV)
                    for p in range(2):
                        poff = 64 * p
                        ys = small.tile([HD + 1, 512], F32, name="ys",
                                        tag="ys", bufs=4)
                        nc.vector.tensor_copy(ys, ps_ys[p][0:HD + 1, :])
                        dn = small.tile([1, 512], F32, name="dn", tag="dn")
                        nc.vector.tensor_copy(dn, ys[HD:HD + 1, :])
                        rcr = small.tile([1, 512], F32, name="rcr",
                                         tag="rcr")
                        nc.vector.reciprocal_approx_fast(rcr, dn)
                        rcb = small.tile([64, 512], F32, name="rcb",
                                         tag="rcb")
                        nc.gpsimd.partition_broadcast(rcb, rcr)
                        nc.vector.tensor_mul(
                            yT_sb[c][poff:poff + 64,
                                     512 * q0:512 * (q0 + 1)],
                            ys[0:HD, :],
                            rcb,
                        )
                    del psys[bi]

            # ---- epilogue (out-DMAs on the idle scalar ring) ----
            pop_until(len(fillers), (3, 3))
            for t in range(12, 16):
                for nb in range(2):
                    proj_item(t, nb, ring=nc.scalar)()

    nc.finalize()
    return nc


def _enable_trace_hooks():
    """Inject antenv.axon_hooks + no-op artifact upload so that
    run_bass_kernel_spmd(trace=True) works under axon in this image."""
    import types
    import antenv

    if "antenv.axon_hooks" not in sys.modules:
        mod = types.ModuleType("antenv.axon_hooks")
        state = {"hook": None}
        mod.set_axon_ntff_profile_hook = lambda h: state.__setitem__("hook", h)
        mod.get_axon_ntff_profile_hook = lambda: state["hook"]
        sys.modules["antenv.axon_hooks"] = mod
        antenv.axon_hooks = mod
        from trn_agent_boot.trn_boot import _ntff_profile_via_ctypes

        mod.set_axon_ntff_profile_hook(
            _ntff_profile_via_ctypes("/opt/axon/libaxon_pjrt.so"))
    from concourse import bass_utils as bu

    bu.upload_artifacts = lambda tmpdir: str(tmpdir)


def kernel(x, w_attn, b_attn, w_proj, b_proj, _trace=False):
    x = np.asarray(x)
    w_attn = np.asarray(w_attn)
    b_attn = np.asarray(b_attn)
    w_proj = np.asarray(w_proj)
    b_proj = np.asarray(b_proj)

    if "nc" not in _cache:
        _cache["nc"] = _build()
    nc = _cache["nc"]

    f16 = np.float16

    in_maps = []
    for core in range(8):
        b, hg = core // 2, core % 2
        qs = slice(hg * DL, (hg + 1) * DL)
        ks = slice(D + hg * DL, D + (hg + 1) * DL)
        vs = slice(2 * D + hg * DL, 2 * D + (hg + 1) * DL)

        xTb = x[b].T  # [D, T]
        # fp16 x, packed [128, tb, k, 512]: row 128k+p, col 512tb+t
        x16 = np.ascontiguousarray(
            xTb.reshape(KCH, 128, TB, 512).transpose(1, 2, 0, 3)
        ).reshape(128, -1).astype(f16)
        # fp8 x, packed [128, tb, i, j, 512]: row 256j+128i+p
        x8 = np.ascontiguousarray(
            xTb.reshape(JCH, 2, 128, TB, 512).transpose(2, 3, 1, 0, 4)
        ).reshape(128, -1).astype(NF8)
        # fp8 w_qk (x32), packed cc-major [128, cc, i, j, m]:
        # element = w[256j+128i+p, 128cc+m]
        wqk_host = np.concatenate(
            [w_attn[:, qs], w_attn[:, ks]], axis=1) * 32.0
        w8 = np.ascontiguousarray(
            wqk_host.reshape(JCH, 2, 128, 8, 128).transpose(2, 3, 1, 0, 4)
        ).reshape(128, -1).astype(NF8)
        bqk_host = np.concatenate(
            [b_attn[qs], b_attn[ks]]).astype(np.float32)
        # fp16 wv packed [128, k, n]: row 128k+p
        wv_host = np.ascontiguousarray(
            w_attn[:, vs].reshape(KCH, 128, DL).transpose(1, 0, 2)
        ).reshape(128, -1).astype(f16)
        # fp16 wp packed [128, c, n]: row 128c+p of the local slice
        wp_host = np.ascontiguousarray(
            w_proj[hg * DL:(hg + 1) * DL, :].reshape(
                DL // 128, 128, D).transpose(1, 0, 2)
        ).reshape(128, -1).astype(f16)

        in_maps.append({
            "xT16": x16,
            "xT8": x8,
            "wqk8": w8,
            "bqk": np.ascontiguousarray(bqk_host.reshape(8, 128).T),
            "wv": wv_host,
            "bv": np.ascontiguousarray(b_attn[vs][None, :]).astype(np.float32),
            "wp": wp_host,
        })

    kwargs = {}
    if _trace:
        _enable_trace_hooks()
        kwargs = dict(trace=True, trace_cores=[0])
    res = run_bass_kernel_spmd(nc, in_maps, core_ids=list(range(8)), **kwargs)

    outp = np.empty((B, T, D), np.float32)
    for b in range(B):
        outp[b] = (res.results[2 * b]["out"].astype(np.float32)
                   + res.results[2 * b + 1]["out"].astype(np.float32))
    outp += b_proj.astype(np.float32)

    if _trace:
        print(f"HW exec time: {res.exec_time_ns} ns")
    return outp
